# revision 3
# baseline (speedup 1.0000x reference)
"""Trainium2 Bass kernel for ProbLinear (Bayesian linear layer, sampled weights).

Computes, in fp32 inputs / float32r matmul precision:
    W    = weight_mu + softplus(weight_rho) * eps_w          [OUT_F, IN_F]
    b    = bias_mu + softplus(bias_rho) * eps_b              [OUT_F]
    out  = x @ W.T + b                                       [TOKENS, OUT_F]

Sharding across 8 NeuronCores: 2-way over tokens x 4-way over out_features.
Each core samples its W slice on-chip, transposes x / W tiles via the PE
(contraction dim must sit on partitions for both matmul operands), and runs
a K-accumulated float32r matmul (full PE rate, ~1.5e-4 rel error).

Self-contained: hardcodes shapes, builds + caches the Bass program, shards
inputs on the host, runs via run_bass_kernel_spmd, reassembles full output.
"""
import numpy as np
from contextlib import ExitStack

import concourse.bass as bass
import concourse.mybir as mybir
import concourse.tile as tile
from concourse.bass_utils import run_bass_kernel_spmd
from concourse.masks import make_identity

# ----------------------------------------------------------------------------
# Workaround for this walrus build: only 1 sem wait per instruction is
# accepted by some codegen paths. After Tile scheduling, hoist excess waits
# onto same-engine NoOps inserted right before the offending instruction.
# ----------------------------------------------------------------------------
_MAX_WAITS = 1


def _split_excess_waits(nc):
    for f in nc.m.functions:
        for bb in f.blocks:
            insts = bb.instructions
            i = 0
            while i < len(insts):
                inst = insts[i]
                si = inst.sync_info
                if si is not None and len(si.on_wait) > _MAX_WAITS:
                    waits = list(si.on_wait)
                    excess, keep = waits[:-_MAX_WAITS], waits[-_MAX_WAITS:]
                    si.on_wait = keep
                    pos = i
                    for j in range(0, len(excess), _MAX_WAITS):
                        chunk = excess[j:j + _MAX_WAITS]
                        nop = mybir.InstNoOp(
                            name=f"{inst.name}-waitsplit-{j}", ins=[], outs=[]
                        )
                        nop.engine = inst.engine
                        nop.sync_info = mybir.SyncInfo(on_wait=chunk, on_update=[])
                        nc.register_instruction(nop, overwrite=True)
                        insts.insert(pos, nop)
                        pos += 1
                        i += 1
                i += 1


if not getattr(tile.TileContext, "_waitsplit_patched", False):
    _orig_exit = tile.TileContext.__exit__

    def _patched_exit(self, exc_type, exc_val, exc_tb):
        res = _orig_exit(self, exc_type, exc_val, exc_tb)
        if exc_type is None:
            _split_excess_waits(self.nc)
        return res

    tile.TileContext.__exit__ = _patched_exit
    tile.TileContext._waitsplit_patched = True

# ----------------------------------------------------------------------------
# Problem shapes / sharding
# ----------------------------------------------------------------------------
TOKENS, IN_F, OUT_F = 8192, 4096, 4096
T_SPLIT, O_SPLIT = 2, 4
N_CORES = T_SPLIT * O_SPLIT

T_C = TOKENS // T_SPLIT          # 4096 tokens per core
O_C = OUT_F // O_SPLIT           # 1024 out features per core
KT = IN_F // 128                 # 32 contraction tiles
TT = T_C // 128                  # 32 token tiles per core
OROWS = O_C // 128               # 8 weight row-tiles per core
KC = 2                           # k-chunks for weight sampling
KCW = IN_F // KC                 # 2048 wide sampling chunks
NB = 512                         # matmul moving free dim (one PSUM bank fp32)
OC = O_C // NB                   # 2 output column chunks per core

F32 = mybir.dt.float32
F32R = mybir.dt.float32r
AF = mybir.ActivationFunctionType


def _build_program():
    nc = bass.Bass()
    x_d = nc.declare_dram_parameter("x", [T_C, IN_F], F32, isOutput=False)
    wmu_d = nc.declare_dram_parameter("wmu", [O_C, IN_F], F32, isOutput=False)
    wrho_d = nc.declare_dram_parameter("wrho", [O_C, IN_F], F32, isOutput=False)
    weps_d = nc.declare_dram_parameter("weps", [O_C, IN_F], F32, isOutput=False)
    bmu_d = nc.declare_dram_parameter("bmu", [O_C], F32, isOutput=False)
    brho_d = nc.declare_dram_parameter("brho", [O_C], F32, isOutput=False)
    beps_d = nc.declare_dram_parameter("beps", [O_C], F32, isOutput=False)
    out_d = nc.declare_dram_parameter("out", [T_C, O_C], F32, isOutput=True)

    with tile.TileContext(nc) as tc, ExitStack() as ctx:
        const = ctx.enter_context(tc.tile_pool(name="const", bufs=1))
        stage = ctx.enter_context(tc.tile_pool(name="stage", bufs=4))
        xtp = ctx.enter_context(tc.tile_pool(name="xtp", bufs=1))
        outp = ctx.enter_context(tc.tile_pool(name="outp", bufs=2))
        tpsum = ctx.enter_context(tc.tile_pool(name="tpsum", bufs=4, space="PSUM"))
        mmpsum = ctx.enter_context(tc.tile_pool(name="mmpsum", bufs=2, space="PSUM"))

        ident = const.tile([128, 128], F32)
        make_identity(nc, ident)

        # ------------------------------------------------------------------
        # Bias: compute on one partition, then broadcast via a K=1 matmul.
        # ------------------------------------------------------------------
        ones = const.tile([1, 128], F32)
        nc.gpsimd.memset(ones[:], 1.0)
        brow_mu = const.tile([1, O_C], F32)
        brow_rho = const.tile([1, O_C], F32)
        brow_eps = const.tile([1, O_C], F32)
        nc.sync.dma_start(brow_mu[:], bmu_d[None, :])
        nc.sync.dma_start(brow_rho[:], brho_d[None, :])
        nc.sync.dma_start(brow_eps[:], beps_d[None, :])
        # softplus(r) = ln(exp(r) + 1); Softplus isn't in this build's tables
        nc.scalar.activation(brow_rho[:], brow_rho[:], AF.Exp)
        nc.scalar.activation(brow_rho[:], brow_rho[:], AF.Ln, bias=1.0)
        nc.vector.tensor_mul(brow_rho[:], brow_rho[:], brow_eps[:])
        nc.vector.tensor_add(brow_rho[:], brow_rho[:], brow_mu[:])
        bias_bc = const.tile([128, O_C], F32)
        for oc in range(OC):
            bps = mmpsum.tile([128, NB], F32, tag="bps")
            nc.tensor.matmul(
                bps[:], ones[:], brow_rho[:, oc * NB:(oc + 1) * NB],
                start=True, stop=True,
            )
            nc.any.tensor_copy(out=bias_bc[:, oc * NB:(oc + 1) * NB], in_=bps[:])

        # ------------------------------------------------------------------
        # Phase 1: sample W slice and build W^T resident in SBUF as f32r.
        # wT[p, ki, o] = W[o, ki*128 + p]
        # ------------------------------------------------------------------
        wT = const.tile([128, KT, O_C], F32R)
        for orow in range(OROWS):
            ro = orow * 128
            for kc in range(KC):
                ci = kc * KCW
                ws = stage.tile([128, KCW], F32, tag="stg")
                eps = stage.tile([128, KCW], F32, tag="stg")
                mu = stage.tile([128, KCW], F32, tag="stg")
                nc.sync.dma_start(ws[:], wrho_d[ro:ro + 128, ci:ci + KCW])
                nc.sync.dma_start(eps[:], weps_d[ro:ro + 128, ci:ci + KCW])
                nc.sync.dma_start(mu[:], wmu_d[ro:ro + 128, ci:ci + KCW])
                nc.scalar.activation(ws[:], ws[:], AF.Exp)
                nc.scalar.activation(ws[:], ws[:], AF.Ln, bias=1.0)
                nc.vector.tensor_mul(ws[:], ws[:], eps[:])
                nc.vector.tensor_add(ws[:], ws[:], mu[:])
                for kt in range(KCW // 128):
                    ki = kc * (KCW // 128) + kt
                    pt = tpsum.tile([128, 128], F32, tag="pt")
                    nc.tensor.transpose(
                        pt[:], ws[:, kt * 128:(kt + 1) * 128], ident[:]
                    )
                    nc.any.tensor_copy(out=wT[:, ki, ro:ro + 128], in_=pt[:])

        # ------------------------------------------------------------------
        # Phase 2: stream x token-tiles, transpose, matmul, bias, store.
        # ------------------------------------------------------------------
        for tt in range(TT):
            rt = tt * 128
            xT = xtp.tile([128, KT, 128], F32R)
            for h in range(KC):
                ci = h * KCW
                xh = stage.tile([128, KCW], F32, tag="stg")
                nc.sync.dma_start(xh[:], x_d[rt:rt + 128, ci:ci + KCW])
                for kt in range(KCW // 128):
                    ki = h * (KCW // 128) + kt
                    pt = tpsum.tile([128, 128], F32, tag="pt")
                    nc.tensor.transpose(
                        pt[:], xh[:, kt * 128:(kt + 1) * 128], ident[:]
                    )
                    nc.any.tensor_copy(out=xT[:, ki], in_=pt[:])
            ot = outp.tile([128, O_C], F32)
            for oc in range(OC):
                ps = mmpsum.tile([128, NB], F32, tag="ps")
                for ki in range(KT):
                    nc.tensor.matmul(
                        ps[:],
                        xT[:, ki],
                        wT[:, ki, oc * NB:(oc + 1) * NB],
                        start=(ki == 0),
                        stop=(ki == KT - 1),
                    )
                nc.vector.tensor_add(
                    ot[:, oc * NB:(oc + 1) * NB], ps[:],
                    bias_bc[:, oc * NB:(oc + 1) * NB],
                )
            nc.sync.dma_start(out_d[rt:rt + 128, :], ot[:])

    return nc


_PROGRAM = None


def kernel(x, weight_mu, weight_rho, bias_mu, bias_rho, eps_w, eps_b):
    global _PROGRAM
    if _PROGRAM is None:
        _PROGRAM = _build_program()
    nc = _PROGRAM

    x = np.ascontiguousarray(np.asarray(x, dtype=np.float32))
    weight_mu = np.ascontiguousarray(np.asarray(weight_mu, dtype=np.float32))
    weight_rho = np.ascontiguousarray(np.asarray(weight_rho, dtype=np.float32))
    eps_w = np.ascontiguousarray(np.asarray(eps_w, dtype=np.float32))
    bias_mu = np.ascontiguousarray(np.asarray(bias_mu, dtype=np.float32))
    bias_rho = np.ascontiguousarray(np.asarray(bias_rho, dtype=np.float32))
    eps_b = np.ascontiguousarray(np.asarray(eps_b, dtype=np.float32))

    in_maps = []
    for c in range(N_CORES):
        ti, oi = c // O_SPLIT, c % O_SPLIT
        ts_, te = ti * T_C, (ti + 1) * T_C
        os_, oe = oi * O_C, (oi + 1) * O_C
        in_maps.append({
            "x": np.ascontiguousarray(x[ts_:te]),
            "wmu": np.ascontiguousarray(weight_mu[os_:oe]),
            "wrho": np.ascontiguousarray(weight_rho[os_:oe]),
            "weps": np.ascontiguousarray(eps_w[os_:oe]),
            "bmu": np.ascontiguousarray(bias_mu[os_:oe]),
            "brho": np.ascontiguousarray(bias_rho[os_:oe]),
            "beps": np.ascontiguousarray(eps_b[os_:oe]),
        })

    res = run_bass_kernel_spmd(nc, in_maps, list(range(N_CORES)))
    kernel.last_results = res

    out = np.empty((TOKENS, OUT_F), dtype=np.float32)
    for c in range(N_CORES):
        ti, oi = c // O_SPLIT, c % O_SPLIT
        out[ti * T_C:(ti + 1) * T_C, oi * O_C:(oi + 1) * O_C] = res.results[c]["out"]
    return out
